# revision 15
# baseline (speedup 1.0000x reference)
"""Trainium2 Bass kernel for nn_ConvolutionalSelfAttention.

Mathematical simplification of the reference:
    v[b,t,o]  = sum_c x[b,t,c] W_attn[o,c]
    s[b,t]    = sum_o v[b,t,o] = sum_c x[b,t,c] * wa[c],   wa = colsum(W_attn)
    y[b,t]    = (s[b,t] + s[b,t-1] + s[b,t-2]) / 3        (zero-padded, causal)
    out[b,t,o]= y[b,t] * wp[o],                            wp = rowsum(W_proj)

Sharding (8 cores, collective-free): each core owns 2048 consecutive
tokens of one batch (b = core//2, half = core%2) plus a 2-token halo.
Every core computes the full wa/wp weight reductions locally from
bf16 copies of W_attn and W_proj^T (host-transposed), so there is no
AllReduce and no cross-core barrier: cores run fully independently.

All HBM streams are bf16 (x, W_attn, W_proj^T, out); the harness gate
is scale-relative absmax, and bf16 keeps the error ~5e-3 << 2e-2.

Per 128-token tile on device:
    s_col = tensor_reduce(tensor_mul(x_tile, wa_bcast/3))      (DVE, two pass;
            tensor_tensor_reduce hangs on this HW path - do not use it)
    y_col = M1^T @ s_col + M2^T @ s_prev_col                   (PE band matmul)
    out   = wp_bcast scaled per-partition by y_col -> bf16     (ACT activation)

Weight reduction: DVE in-place add chain over 16 row-tiles -> [128, C] acc,
one PE ones-matmul colsum -> [1, C], PE K=1 broadcast -> [128, C].
All DMAs ride gpsimd SWDGE (HWDGE dma_start also failed on this path).
"""

import numpy as np
from contextlib import ExitStack

B, T, C = 4, 4096, 2048
N_CORES = 8
T_LOC = (B * T) // N_CORES      # 2048 tokens per core
P = 128
NT = T_LOC // P                 # 16 token tiles per core
GW = C // P                     # 16 row-tiles in a [C, C] matrix
WCH = 8                         # row-tiles per weight DMA chunk (4 MB bf16)
NWC = GW // WCH                 # 2 chunks per matrix
XCH = 4                         # token tiles per x/out DMA chunk (2 MB bf16)
NXC = NT // XCH                 # 4 chunks
THIRD = 1.0 / 3.0

_BUILT = {}


def _band_consts():
    # lhsT layout: y[m] = sum_k M[k, m] * s[k]
    m1 = np.zeros((P, P), np.float32)
    for m in range(P):
        m1[max(0, m - 2):m + 1, m] = 1.0
    # carry from previous tile's last two tokens (partitions 126/127)
    m2 = np.zeros((P, P), np.float32)
    m2[126, 0] = 1.0
    m2[127, 0] = 1.0
    m2[127, 1] = 1.0
    # carry for tile 0: halo column stores s[-2], s[-1] at partitions 0/1
    m2h = np.zeros((P, P), np.float32)
    m2h[0, 0] = 1.0
    m2h[1, 0] = 1.0
    m2h[1, 1] = 1.0
    return m1, m2, m2h


def _build_nc():
    import concourse.bass as bass
    import concourse.tile as tile
    from concourse import bacc, mybir

    f32 = mybir.dt.float32
    bf16 = mybir.dt.bfloat16
    AF = mybir.ActivationFunctionType
    ALU = mybir.AluOpType

    nc = bacc.Bacc("TRN2", target_bir_lowering=False, debug=False,
                   num_devices=N_CORES)

    x_shard = nc.dram_tensor("x_shard", [T_LOC, C], bf16, kind="ExternalInput")
    x_halo = nc.dram_tensor("x_halo", [2, C], bf16, kind="ExternalInput")
    w_attn = nc.dram_tensor("w_attn", [C, C], bf16, kind="ExternalInput")
    w_projT = nc.dram_tensor("w_projT", [C, C], bf16, kind="ExternalInput")
    out = nc.dram_tensor("out", [T_LOC, C], bf16, kind="ExternalOutput")

    m1_c = nc.dram_tensor("m1_band", [P, P], f32, kind="ExternalInput")
    m2_c = nc.dram_tensor("m2_band", [P, P], f32, kind="ExternalInput")
    m2h_c = nc.dram_tensor("m2h_band", [P, P], f32, kind="ExternalInput")
    ones_k_c = nc.dram_tensor("ones_k", [P, 1], bf16, kind="ExternalInput")
    ones_m_c = nc.dram_tensor("ones_m", [1, P], f32, kind="ExternalInput")

    with tile.TileContext(nc) as tc, ExitStack() as ctx:
        cpool = ctx.enter_context(tc.tile_pool(name="const", bufs=1))
        wchunk = ctx.enter_context(tc.tile_pool(name="wchunk", bufs=2))
        xchunk = ctx.enter_context(tc.tile_pool(name="xchunk", bufs=3))
        opool = ctx.enter_context(tc.tile_pool(name="o", bufs=2))
        scratch = ctx.enter_context(tc.tile_pool(name="scratch", bufs=2))
        spool = ctx.enter_context(tc.tile_pool(name="small", bufs=4))
        psA = ctx.enter_context(tc.tile_pool(name="psA", bufs=2, space="PSUM"))
        psB = ctx.enter_context(tc.tile_pool(name="psB", bufs=2, space="PSUM"))
        ypsum = ctx.enter_context(tc.tile_pool(name="ypsum", bufs=4, space="PSUM"))

        def stream_weight_adds(dram_t):
            """stream a [C, C] bf16 matrix, add 16 row-tiles -> [P, C] acc."""
            acc = cpool.tile([P, C], bf16, tag="wacc_" + dram_t.name)
            for c in range(NWC):
                wc = wchunk.tile([P, WCH * C], bf16, tag="wc")
                nc.gpsimd.dma_start(
                    wc[:].rearrange("p (h c) -> p h c", h=WCH),
                    dram_t.ap()[c * WCH * P:(c + 1) * WCH * P, :]
                    .rearrange("(h p) c -> p h c", p=P))
                for h in range(WCH):
                    if c == 0 and h == 0:
                        continue
                    if c == 0 and h == 1:
                        nc.vector.tensor_tensor(
                            acc[:], wc[:, 0:C], wc[:, C:2 * C], ALU.add)
                    else:
                        nc.vector.tensor_tensor(
                            acc[:], acc[:], wc[:, h * C:(h + 1) * C], ALU.add)
            return acc

        def colsum_bcast(acc, name, scale):
            """[P, C] acc -> colsum row -> bf16 [P, C] broadcast * scale."""
            row = cpool.tile([1, C], f32, tag="wrow_" + name)
            for j in range(C // 512):
                pj = psA.tile([1, 512], f32)
                nc.tensor.matmul(pj[:], lhsT=ones_k[:],
                                 rhs=acc[:, j * 512:(j + 1) * 512],
                                 start=True, stop=True)
                nc.scalar.copy(row[0:1, j * 512:(j + 1) * 512], pj[:])
            bcast = cpool.tile([P, C], bf16, tag="wbc_" + name)
            for j in range(C // 512):
                bp = psB.tile([P, 512], f32)
                nc.tensor.matmul(bp[:], lhsT=ones_m[:],
                                 rhs=row[0:1, j * 512:(j + 1) * 512],
                                 start=True, stop=True)
                if scale == 1.0:
                    nc.scalar.copy(bcast[:, j * 512:(j + 1) * 512], bp[:])
                else:
                    nc.scalar.mul(bcast[:, j * 512:(j + 1) * 512], bp[:], scale)
            return bcast

        # W streams issue FIRST so their descriptors lead the gpsimd SWDGE
        # queue; the tiny const DMAs generate while the 4MB chunks transfer.
        m1_sb = cpool.tile([P, P], f32)
        m2_sb = cpool.tile([P, P], f32)
        m2h_sb = cpool.tile([P, P], f32)
        ones_k = cpool.tile([P, 1], bf16)
        ones_m = cpool.tile([1, P], f32)
        ht = cpool.tile([2, C], bf16)

        acc_a = stream_weight_adds(w_attn)
        nc.gpsimd.dma_start(ones_k[:], ones_k_c.ap())
        nc.gpsimd.dma_start(ones_m[:], ones_m_c.ap())
        nc.gpsimd.dma_start(m1_sb[:], m1_c.ap())
        nc.gpsimd.dma_start(m2_sb[:], m2_c.ap())
        nc.gpsimd.dma_start(m2h_sb[:], m2h_c.ap())
        nc.gpsimd.dma_start(ht[:], x_halo.ap())
        acc_p = stream_weight_adds(w_projT)
        wa_bcast = colsum_bcast(acc_a, "wa", THIRD)   # wa/3, bf16 [P, C]
        wp_bcast = colsum_bcast(acc_p, "wp", 1.0)     # wp,   bf16 [P, C]

        # ---- halo s values: partitions 0/1 of a zeroed [P, 1] column
        s_halo = cpool.tile([P, 1], f32)
        nc.vector.memset(s_halo[:], 0.0)
        scr_h = scratch.tile([P, C], bf16, tag="scr")
        scr2_h = scratch.tile([P, C], bf16, tag="scr2")
        nc.vector.tensor_mul(scr_h[0:2, :], ht[0:2, :], wa_bcast[0:2, :])
        # free-dim sum via ACT accum_out (DVE tensor_reduce is capped at 1x
        # and would bottleneck; the copy output is discarded)
        nc.scalar.activation(scr2_h[0:2, :], scr_h[0:2, :], AF.Copy,
                             accum_out=s_halo[0:2, 0:1])

        # ---- main loop: stream 16 token tiles in 4 chunks
        s_prev = s_halo
        for ch in range(NXC):
            xc = xchunk.tile([P, XCH * C], bf16, tag="xc")
            nc.gpsimd.dma_start(
                xc[:].rearrange("p (h c) -> p h c", h=XCH),
                x_shard.ap()[ch * XCH * P:(ch + 1) * XCH * P, :]
                .rearrange("(h p) c -> p h c", p=P))
            oc = opool.tile([P, XCH * C], bf16, tag="oc")
            for h in range(XCH):
                i = ch * XCH + h
                scr = scratch.tile([P, C], bf16, tag="scr")
                scr2 = scratch.tile([P, C], bf16, tag="scr2")
                s_cur = spool.tile([P, 1], f32, tag="scol")
                nc.vector.tensor_mul(scr[:], xc[:, h * C:(h + 1) * C],
                                     wa_bcast[:])
                nc.scalar.activation(scr2[:], scr[:], AF.Copy,
                                     accum_out=s_cur[:])
                yp = ypsum.tile([P, 1], f32)
                nc.tensor.matmul(yp[:], lhsT=m1_sb[:], rhs=s_cur[:],
                                 start=True, stop=False)
                carry = m2h_sb if i == 0 else m2_sb
                nc.tensor.matmul(yp[:], lhsT=carry[:], rhs=s_prev[:],
                                 start=False, stop=True)
                ysb = spool.tile([P, 1], f32, tag="ycol")
                nc.vector.tensor_copy(ysb[:], yp[:])
                # out tile on DVE tensor_scalar (4x bf16 tier, ~0.7us) to keep
                # ACT free for the accum reductions
                nc.vector.tensor_scalar_mul(oc[:, h * C:(h + 1) * C],
                                            wp_bcast[:], ysb[:, 0:1])
                s_prev = s_cur
                if h % 2 == 1:
                    # drain finished half-chunks early (1MB writes) so the
                    # out stream overlaps the tail of the compute chain
                    g0 = ch * XCH + h - 1
                    nc.gpsimd.dma_start(
                        out.ap()[g0 * P:(g0 + 2) * P, :]
                        .rearrange("(h p) c -> p h c", p=P),
                        oc[:, (h - 1) * C:(h + 1) * C]
                        .rearrange("p (h c) -> p h c", h=2))

    nc.compile()
    return nc


def _get_nc():
    if "nc" not in _BUILT:
        _BUILT["nc"] = _build_nc()
    return _BUILT["nc"]


def make_in_maps(x, W_attn, W_proj):
    import ml_dtypes
    bf = ml_dtypes.bfloat16
    x = np.asarray(x, dtype=np.float32)
    wa_bf = np.ascontiguousarray(np.asarray(W_attn, dtype=np.float32)).astype(bf)
    wpT_bf = np.ascontiguousarray(
        np.asarray(W_proj, dtype=np.float32).T).astype(bf)
    x_bf = x.astype(bf)
    m1_np, m2_np, m2h_np = _band_consts()
    consts = {
        "m1_band": m1_np, "m2_band": m2_np, "m2h_band": m2h_np,
        "ones_k": np.ones((P, 1), bf),
        "ones_m": np.ones((1, P), np.float32),
        "w_attn": wa_bf,
        "w_projT": wpT_bf,
    }
    in_maps = []
    for k in range(N_CORES):
        b, h = divmod(k, 2)
        t0 = h * T_LOC
        if h == 0:
            halo = np.zeros((2, C), bf)
        else:
            halo = np.ascontiguousarray(x_bf[b, t0 - 2:t0, :])
        in_maps.append({
            "x_shard": np.ascontiguousarray(x_bf[b, t0:t0 + T_LOC, :]),
            "x_halo": halo,
            **consts,
        })
    return in_maps


def assemble(results):
    out_full = np.empty((B, T, C), np.float32)
    for k in range(N_CORES):
        b, h = divmod(k, 2)
        t0 = h * T_LOC
        out_full[b, t0:t0 + T_LOC, :] = np.asarray(
            results[k]["out"], dtype=np.float32)
    return out_full


def kernel(x, W_attn, W_proj):
    from concourse.bass_utils import run_bass_kernel_spmd

    nc = _get_nc()
    in_maps = make_in_maps(x, W_attn, W_proj)
    res = run_bass_kernel_spmd(nc, in_maps, list(range(N_CORES)))
    return assemble(res.results)


# revision 20
# speedup vs baseline: 1.0484x; 1.0484x over previous
"""Trainium2 Bass kernel for nn_ConvolutionalSelfAttention.

Mathematical simplification of the reference:
    v[b,t,o]  = sum_c x[b,t,c] W_attn[o,c]
    s[b,t]    = sum_o v[b,t,o] = sum_c x[b,t,c] * wa[c],   wa = colsum(W_attn)
    y[b,t]    = (s[b,t] + s[b,t-1] + s[b,t-2]) / 3        (zero-padded, causal)
    out[b,t,o]= y[b,t] * wp[o],                            wp = rowsum(W_proj)

Sharding (8 cores, collective-free): each core owns 2048 consecutive
tokens of one batch (b = core//2, half = core%2) plus a 2-token halo.
Every core computes the full wa/wp weight reductions locally from
bf16 copies of W_attn and W_proj^T (host-transposed), so there is no
AllReduce and no cross-core barrier: cores run fully independently.

All HBM streams are bf16 (x, W_attn, W_proj^T, out); the harness gate
is scale-relative absmax, and bf16 keeps the error ~5e-3 << 2e-2.

Per 128-token tile on device:
    s_col = tensor_reduce(tensor_mul(x_tile, wa_bcast/3))      (DVE, two pass;
            tensor_tensor_reduce hangs on this HW path - do not use it)
    y_col = M1^T @ s_col + M2^T @ s_prev_col                   (PE band matmul)
    out   = wp_bcast scaled per-partition by y_col -> bf16     (ACT activation)

Weight reduction: DVE in-place add chain over 16 row-tiles -> [128, C] acc,
one PE ones-matmul colsum -> [1, C], PE K=1 broadcast -> [128, C].
All DMAs ride gpsimd SWDGE (HWDGE dma_start also failed on this path).
"""

import numpy as np
from contextlib import ExitStack

B, T, C = 4, 4096, 2048
N_CORES = 8
T_LOC = (B * T) // N_CORES      # 2048 tokens per core
P = 128
NT = T_LOC // P                 # 16 token tiles per core
GW = C // P                     # 16 row-tiles in a [C, C] matrix
WCH = 4                         # row-tiles per weight DMA chunk (2 MB bf16)
NWC = GW // WCH                 # 4 chunks per matrix
XCH = 4                         # token tiles per x/out DMA chunk (2 MB bf16)
NXC = NT // XCH                 # 4 chunks
THIRD = 1.0 / 3.0

_BUILT = {}


def _band_consts():
    # lhsT layout: y[m] = sum_k M[k, m] * s[k]
    m1 = np.zeros((P, P), np.float32)
    for m in range(P):
        m1[max(0, m - 2):m + 1, m] = 1.0
    # carry from previous tile's last two tokens (partitions 126/127)
    m2 = np.zeros((P, P), np.float32)
    m2[126, 0] = 1.0
    m2[127, 0] = 1.0
    m2[127, 1] = 1.0
    # carry for tile 0: halo column stores s[-2], s[-1] at partitions 0/1
    m2h = np.zeros((P, P), np.float32)
    m2h[0, 0] = 1.0
    m2h[1, 0] = 1.0
    m2h[1, 1] = 1.0
    return m1, m2, m2h


def _build_nc():
    import concourse.bass as bass
    import concourse.tile as tile
    from concourse import bacc, mybir

    f32 = mybir.dt.float32
    bf16 = mybir.dt.bfloat16
    AF = mybir.ActivationFunctionType
    ALU = mybir.AluOpType

    nc = bacc.Bacc("TRN2", target_bir_lowering=False, debug=False,
                   num_devices=N_CORES)

    x_shard = nc.dram_tensor("x_shard", [T_LOC, C], bf16, kind="ExternalInput")
    x_halo = nc.dram_tensor("x_halo", [2, C], bf16, kind="ExternalInput")
    w_attn = nc.dram_tensor("w_attn", [C, C], bf16, kind="ExternalInput")
    w_projT = nc.dram_tensor("w_projT", [C, C], bf16, kind="ExternalInput")
    out = nc.dram_tensor("out", [T_LOC, C], bf16, kind="ExternalOutput")

    m1_c = nc.dram_tensor("m1_band", [P, P], f32, kind="ExternalInput")
    m2_c = nc.dram_tensor("m2_band", [P, P], f32, kind="ExternalInput")
    m2h_c = nc.dram_tensor("m2h_band", [P, P], f32, kind="ExternalInput")
    ones_k_c = nc.dram_tensor("ones_k", [P, 1], bf16, kind="ExternalInput")
    ones_m_c = nc.dram_tensor("ones_m", [1, P], f32, kind="ExternalInput")

    with tile.TileContext(nc) as tc, ExitStack() as ctx:
        cpool = ctx.enter_context(tc.tile_pool(name="const", bufs=1))
        wchunk = ctx.enter_context(tc.tile_pool(name="wchunk", bufs=3))
        xchunk = ctx.enter_context(tc.tile_pool(name="xchunk", bufs=4))
        opool = ctx.enter_context(tc.tile_pool(name="o", bufs=2))
        scratch = ctx.enter_context(tc.tile_pool(name="scratch", bufs=2))
        spool = ctx.enter_context(tc.tile_pool(name="small", bufs=4))
        psA = ctx.enter_context(tc.tile_pool(name="psA", bufs=2, space="PSUM"))
        psB = ctx.enter_context(tc.tile_pool(name="psB", bufs=2, space="PSUM"))
        ypsum = ctx.enter_context(tc.tile_pool(name="ypsum", bufs=4, space="PSUM"))

        def stream_weight_adds(dram_t):
            """stream a [C, C] bf16 matrix, add 16 row-tiles -> [P, C] acc."""
            acc = cpool.tile([P, C], bf16, tag="wacc_" + dram_t.name)
            for c in range(NWC):
                wc = wchunk.tile([P, WCH * C], bf16, tag="wc")
                # p-major row grouping: WCH consecutive DRAM rows land in one
                # partition as one contiguous 16KB descriptor (colsum is row-
                # order independent, so the mapping does not matter)
                nc.gpsimd.dma_start(
                    wc[:].rearrange("p (h c) -> p h c", h=WCH),
                    dram_t.ap()[c * WCH * P:(c + 1) * WCH * P, :]
                    .rearrange("(p h) c -> p h c", h=WCH))
                for h in range(WCH):
                    if c == 0 and h == 0:
                        continue
                    if c == 0 and h == 1:
                        nc.vector.tensor_tensor(
                            acc[:], wc[:, 0:C], wc[:, C:2 * C], ALU.add)
                    else:
                        nc.vector.tensor_tensor(
                            acc[:], acc[:], wc[:, h * C:(h + 1) * C], ALU.add)
            return acc

        def colsum_bcast(acc, name, scale):
            """[P, C] acc -> colsum row -> bf16 [P, C] broadcast * scale."""
            row = cpool.tile([1, C], f32, tag="wrow_" + name)
            for j in range(C // 512):
                pj = psA.tile([1, 512], f32)
                nc.tensor.matmul(pj[:], lhsT=ones_k[:],
                                 rhs=acc[:, j * 512:(j + 1) * 512],
                                 start=True, stop=True)
                nc.scalar.copy(row[0:1, j * 512:(j + 1) * 512], pj[:])
            bcast = cpool.tile([P, C], bf16, tag="wbc_" + name)
            for j in range(C // 512):
                bp = psB.tile([P, 512], f32)
                nc.tensor.matmul(bp[:], lhsT=ones_m[:],
                                 rhs=row[0:1, j * 512:(j + 1) * 512],
                                 start=True, stop=True)
                if scale == 1.0:
                    nc.scalar.copy(bcast[:, j * 512:(j + 1) * 512], bp[:])
                else:
                    nc.scalar.mul(bcast[:, j * 512:(j + 1) * 512], bp[:], scale)
            return bcast

        # W streams issue FIRST so their descriptors lead the gpsimd SWDGE
        # queue; the tiny const DMAs generate while the 4MB chunks transfer.
        m1_sb = cpool.tile([P, P], f32)
        m2_sb = cpool.tile([P, P], f32)
        m2h_sb = cpool.tile([P, P], f32)
        ones_k = cpool.tile([P, 1], bf16)
        ones_m = cpool.tile([1, P], f32)
        ht = cpool.tile([2, C], bf16)

        acc_a = stream_weight_adds(w_attn)
        nc.gpsimd.dma_start(ones_k[:], ones_k_c.ap())
        nc.gpsimd.dma_start(ones_m[:], ones_m_c.ap())
        nc.gpsimd.dma_start(m1_sb[:], m1_c.ap())
        nc.gpsimd.dma_start(m2_sb[:], m2_c.ap())
        nc.gpsimd.dma_start(m2h_sb[:], m2h_c.ap())
        nc.gpsimd.dma_start(ht[:], x_halo.ap())
        acc_p = stream_weight_adds(w_projT)
        wa_bcast = colsum_bcast(acc_a, "wa", THIRD)   # wa/3, bf16 [P, C]
        wp_bcast = colsum_bcast(acc_p, "wp", 1.0)     # wp,   bf16 [P, C]

        # ---- halo s values: partitions 0/1 of a zeroed [P, 1] column
        s_halo = cpool.tile([P, 1], f32)
        nc.vector.memset(s_halo[:], 0.0)
        scr_h = scratch.tile([P, C], bf16, tag="scr")
        scr2_h = scratch.tile([P, C], bf16, tag="scr2")
        nc.vector.tensor_mul(scr_h[0:2, :], ht[0:2, :], wa_bcast[0:2, :])
        # free-dim sum via ACT accum_out (DVE tensor_reduce is capped at 1x
        # and would bottleneck; the copy output is discarded)
        nc.scalar.activation(scr2_h[0:2, :], scr_h[0:2, :], AF.Copy,
                             accum_out=s_halo[0:2, 0:1])

        # ---- main loop: stream 16 token tiles in 4 chunks
        s_prev = s_halo
        for ch in range(NXC):
            xc = xchunk.tile([P, XCH * C], bf16, tag="xc")
            nc.gpsimd.dma_start(
                xc[:].rearrange("p (h c) -> p h c", h=XCH),
                x_shard.ap()[ch * XCH * P:(ch + 1) * XCH * P, :]
                .rearrange("(h p) c -> p h c", p=P))
            oc = opool.tile([P, XCH * C], bf16, tag="oc")
            for h in range(XCH):
                i = ch * XCH + h
                scr = scratch.tile([P, C], bf16, tag="scr")
                scr2 = scratch.tile([P, C], bf16, tag="scr2")
                s_cur = spool.tile([P, 1], f32, tag="scol")
                nc.vector.tensor_mul(scr[:], xc[:, h * C:(h + 1) * C],
                                     wa_bcast[:])
                nc.scalar.activation(scr2[:], scr[:], AF.Copy,
                                     accum_out=s_cur[:])
                yp = ypsum.tile([P, 1], f32)
                nc.tensor.matmul(yp[:], lhsT=m1_sb[:], rhs=s_cur[:],
                                 start=True, stop=False)
                carry = m2h_sb if i == 0 else m2_sb
                nc.tensor.matmul(yp[:], lhsT=carry[:], rhs=s_prev[:],
                                 start=False, stop=True)
                ysb = spool.tile([P, 1], f32, tag="ycol")
                nc.vector.tensor_copy(ysb[:], yp[:])
                # out tile on DVE tensor_scalar (4x bf16 tier, ~0.7us) to keep
                # ACT free for the accum reductions
                nc.vector.tensor_scalar_mul(oc[:, h * C:(h + 1) * C],
                                            wp_bcast[:], ysb[:, 0:1])
                s_prev = s_cur
            nc.gpsimd.dma_start(
                out.ap()[ch * XCH * P:(ch + 1) * XCH * P, :]
                .rearrange("(h p) c -> p h c", p=P),
                oc[:].rearrange("p (h c) -> p h c", h=XCH))

    nc.compile()
    return nc


def _get_nc():
    if "nc" not in _BUILT:
        _BUILT["nc"] = _build_nc()
    return _BUILT["nc"]


def make_in_maps(x, W_attn, W_proj):
    import ml_dtypes
    bf = ml_dtypes.bfloat16
    x = np.asarray(x, dtype=np.float32)
    wa_bf = np.ascontiguousarray(np.asarray(W_attn, dtype=np.float32)).astype(bf)
    wpT_bf = np.ascontiguousarray(
        np.asarray(W_proj, dtype=np.float32).T).astype(bf)
    x_bf = x.astype(bf)
    m1_np, m2_np, m2h_np = _band_consts()
    consts = {
        "m1_band": m1_np, "m2_band": m2_np, "m2h_band": m2h_np,
        "ones_k": np.ones((P, 1), bf),
        "ones_m": np.ones((1, P), np.float32),
        "w_attn": wa_bf,
        "w_projT": wpT_bf,
    }
    in_maps = []
    for k in range(N_CORES):
        b, h = divmod(k, 2)
        t0 = h * T_LOC
        if h == 0:
            halo = np.zeros((2, C), bf)
        else:
            halo = np.ascontiguousarray(x_bf[b, t0 - 2:t0, :])
        in_maps.append({
            "x_shard": np.ascontiguousarray(x_bf[b, t0:t0 + T_LOC, :]),
            "x_halo": halo,
            **consts,
        })
    return in_maps


def assemble(results):
    out_full = np.empty((B, T, C), np.float32)
    for k in range(N_CORES):
        b, h = divmod(k, 2)
        t0 = h * T_LOC
        out_full[b, t0:t0 + T_LOC, :] = np.asarray(
            results[k]["out"], dtype=np.float32)
    return out_full


def kernel(x, W_attn, W_proj):
    from concourse.bass_utils import run_bass_kernel_spmd

    nc = _get_nc()
    in_maps = make_in_maps(x, W_attn, W_proj)
    res = run_bass_kernel_spmd(nc, in_maps, list(range(N_CORES)))
    return assemble(res.results)


# revision 21
# speedup vs baseline: 1.0881x; 1.0379x over previous
"""Trainium2 Bass kernel for nn_ConvolutionalSelfAttention.

Mathematical simplification of the reference:
    v[b,t,o]  = sum_c x[b,t,c] W_attn[o,c]
    s[b,t]    = sum_o v[b,t,o] = sum_c x[b,t,c] * wa[c],   wa = colsum(W_attn)
    y[b,t]    = (s[b,t] + s[b,t-1] + s[b,t-2]) / 3        (zero-padded, causal)
    out[b,t,o]= y[b,t] * wp[o],                            wp = rowsum(W_proj)

Sharding (8 cores, collective-free): each core owns 2048 consecutive
tokens of one batch (b = core//2, half = core%2) plus a 2-token halo.
Every core computes the full wa/wp weight reductions locally from
bf16 copies of W_attn and W_proj^T (host-transposed), so there is no
AllReduce and no cross-core barrier: cores run fully independently.

All HBM streams are bf16 (x, W_attn, W_proj^T, out); the harness gate
is scale-relative absmax, and bf16 keeps the error ~5e-3 << 2e-2.

Per 128-token tile on device:
    s_col = tensor_reduce(tensor_mul(x_tile, wa_bcast/3))      (DVE, two pass;
            tensor_tensor_reduce hangs on this HW path - do not use it)
    y_col = M1^T @ s_col + M2^T @ s_prev_col                   (PE band matmul)
    out   = wp_bcast scaled per-partition by y_col -> bf16     (ACT activation)

Weight reduction: DVE in-place add chain over 16 row-tiles -> [128, C] acc,
one PE ones-matmul colsum -> [1, C], PE K=1 broadcast -> [128, C].
All DMAs ride gpsimd SWDGE (HWDGE dma_start also failed on this path).
"""

import numpy as np
from contextlib import ExitStack

B, T, C = 4, 4096, 2048
N_CORES = 8
T_LOC = (B * T) // N_CORES      # 2048 tokens per core
P = 128
NT = T_LOC // P                 # 16 token tiles per core
GW = C // P                     # 16 row-tiles in a [C, C] matrix
WCH = 4                         # row-tiles per weight DMA chunk (2 MB bf16)
NWC = GW // WCH                 # 4 chunks per matrix
XCH = 4                         # token tiles per x/out DMA chunk (2 MB bf16)
NXC = NT // XCH                 # 4 chunks
THIRD = 1.0 / 3.0

_BUILT = {}


def _band_consts():
    # lhsT layout: y[m] = sum_k M[k, m] * s[k]
    m1 = np.zeros((P, P), np.float32)
    for m in range(P):
        m1[max(0, m - 2):m + 1, m] = 1.0
    # carry from previous tile's last two tokens (partitions 126/127)
    m2 = np.zeros((P, P), np.float32)
    m2[126, 0] = 1.0
    m2[127, 0] = 1.0
    m2[127, 1] = 1.0
    # carry for tile 0: halo column stores s[-2], s[-1] at partitions 0/1
    m2h = np.zeros((P, P), np.float32)
    m2h[0, 0] = 1.0
    m2h[1, 0] = 1.0
    m2h[1, 1] = 1.0
    return m1, m2, m2h


def _build_nc():
    import concourse.bass as bass
    import concourse.tile as tile
    from concourse import bacc, mybir

    f32 = mybir.dt.float32
    bf16 = mybir.dt.bfloat16
    AF = mybir.ActivationFunctionType
    ALU = mybir.AluOpType

    nc = bacc.Bacc("TRN2", target_bir_lowering=False, debug=False,
                   num_devices=N_CORES)

    x_shard = nc.dram_tensor("x_shard", [T_LOC, C], bf16, kind="ExternalInput")
    x_halo = nc.dram_tensor("x_halo", [2, C], bf16, kind="ExternalInput")
    w_attn = nc.dram_tensor("w_attn", [C, C], bf16, kind="ExternalInput")
    w_projT = nc.dram_tensor("w_projT", [C, C], bf16, kind="ExternalInput")
    out = nc.dram_tensor("out", [T_LOC, C], bf16, kind="ExternalOutput")

    m1_c = nc.dram_tensor("m1_band", [P, P], f32, kind="ExternalInput")
    m2_c = nc.dram_tensor("m2_band", [P, P], f32, kind="ExternalInput")
    m2h_c = nc.dram_tensor("m2h_band", [P, P], f32, kind="ExternalInput")
    ones_k_c = nc.dram_tensor("ones_k", [P, 1], bf16, kind="ExternalInput")
    ones_m_c = nc.dram_tensor("ones_m", [1, P], f32, kind="ExternalInput")

    with tile.TileContext(nc) as tc, ExitStack() as ctx:
        cpool = ctx.enter_context(tc.tile_pool(name="const", bufs=1))
        wchunk = ctx.enter_context(tc.tile_pool(name="wchunk", bufs=3))
        xchunk = ctx.enter_context(tc.tile_pool(name="xchunk", bufs=4))
        opool = ctx.enter_context(tc.tile_pool(name="o", bufs=2))
        scratch = ctx.enter_context(tc.tile_pool(name="scratch", bufs=2))
        spool = ctx.enter_context(tc.tile_pool(name="small", bufs=4))
        psA = ctx.enter_context(tc.tile_pool(name="psA", bufs=2, space="PSUM"))
        psB = ctx.enter_context(tc.tile_pool(name="psB", bufs=2, space="PSUM"))
        ypsum = ctx.enter_context(tc.tile_pool(name="ypsum", bufs=4, space="PSUM"))

        def stream_weight_adds(dram_t):
            """stream a [C, C] bf16 matrix, add 16 row-tiles -> [P, C] acc."""
            acc = cpool.tile([P, C], bf16, tag="wacc_" + dram_t.name)
            for c in range(NWC):
                wc = wchunk.tile([P, WCH * C], bf16, tag="wc")
                # p-major row grouping: WCH consecutive DRAM rows land in one
                # partition as one contiguous 16KB descriptor (colsum is row-
                # order independent, so the mapping does not matter)
                nc.gpsimd.dma_start(
                    wc[:].rearrange("p (h c) -> p h c", h=WCH),
                    dram_t.ap()[c * WCH * P:(c + 1) * WCH * P, :]
                    .rearrange("(p h) c -> p h c", h=WCH))
                for h in range(WCH):
                    if c == 0 and h == 0:
                        continue
                    if c == 0 and h == 1:
                        nc.vector.tensor_tensor(
                            acc[:], wc[:, 0:C], wc[:, C:2 * C], ALU.add)
                    else:
                        nc.vector.tensor_tensor(
                            acc[:], acc[:], wc[:, h * C:(h + 1) * C], ALU.add)
            return acc

        def colsum_bcast(acc, name, scale):
            """[P, C] acc -> colsum row -> bf16 [P, C] broadcast * scale."""
            row = cpool.tile([1, C], f32, tag="wrow_" + name)
            for j in range(C // 512):
                pj = psA.tile([1, 512], f32)
                nc.tensor.matmul(pj[:], lhsT=ones_k[:],
                                 rhs=acc[:, j * 512:(j + 1) * 512],
                                 start=True, stop=True)
                nc.scalar.copy(row[0:1, j * 512:(j + 1) * 512], pj[:])
            bcast = cpool.tile([P, C], bf16, tag="wbc_" + name)
            for j in range(C // 512):
                bp = psB.tile([P, 512], f32)
                nc.tensor.matmul(bp[:], lhsT=ones_m[:],
                                 rhs=row[0:1, j * 512:(j + 1) * 512],
                                 start=True, stop=True)
                if scale == 1.0:
                    nc.scalar.copy(bcast[:, j * 512:(j + 1) * 512], bp[:])
                else:
                    nc.scalar.mul(bcast[:, j * 512:(j + 1) * 512], bp[:], scale)
            return bcast

        # W streams issue FIRST so their descriptors lead the gpsimd SWDGE
        # queue; the tiny const DMAs generate while the 4MB chunks transfer.
        m1_sb = cpool.tile([P, P], f32)
        m2_sb = cpool.tile([P, P], f32)
        m2h_sb = cpool.tile([P, P], f32)
        ones_k = cpool.tile([P, 1], bf16)
        ones_m = cpool.tile([1, P], f32)
        ht = cpool.tile([2, C], bf16)

        acc_a = stream_weight_adds(w_attn)
        nc.gpsimd.dma_start(ones_k[:], ones_k_c.ap())
        nc.gpsimd.dma_start(ones_m[:], ones_m_c.ap())
        nc.gpsimd.dma_start(m1_sb[:], m1_c.ap())
        nc.gpsimd.dma_start(m2_sb[:], m2_c.ap())
        nc.gpsimd.dma_start(m2h_sb[:], m2h_c.ap())
        nc.gpsimd.dma_start(ht[:], x_halo.ap())
        acc_p = stream_weight_adds(w_projT)
        wa_bcast = colsum_bcast(acc_a, "wa", THIRD)   # wa/3, bf16 [P, C]
        wp_bcast = colsum_bcast(acc_p, "wp", 1.0)     # wp,   bf16 [P, C]

        # ---- halo s values: partitions 0/1 of a zeroed [P, 1] column
        s_halo = cpool.tile([P, 1], f32)
        nc.vector.memset(s_halo[:], 0.0)
        scr_h = scratch.tile([P, C], bf16, tag="scr")
        scr2_h = scratch.tile([P, C], bf16, tag="scr2")
        nc.vector.tensor_mul(scr_h[0:2, :], ht[0:2, :], wa_bcast[0:2, :])
        # free-dim sum via ACT accum_out (DVE tensor_reduce is capped at 1x
        # and would bottleneck; the copy output is discarded)
        nc.scalar.activation(scr2_h[0:2, :], scr_h[0:2, :], AF.Copy,
                             accum_out=s_halo[0:2, 0:1])

        # ---- main loop: stream 16 token tiles in 4 chunks
        s_prev = s_halo
        for ch in range(NXC):
            xc = xchunk.tile([P, XCH * C], bf16, tag="xc")
            nc.gpsimd.dma_start(
                xc[:].rearrange("p (h c) -> p h c", h=XCH),
                x_shard.ap()[ch * XCH * P:(ch + 1) * XCH * P, :]
                .rearrange("(h p) c -> p h c", p=P))
            oc = opool.tile([P, XCH * C], bf16, tag="oc")
            for h in range(XCH):
                i = ch * XCH + h
                scr = scratch.tile([P, C], bf16, tag="scr")
                scr2 = scratch.tile([P, C], bf16, tag="scr2")
                s_cur = spool.tile([P, 1], f32, tag="scol")
                nc.vector.tensor_mul(scr[:], xc[:, h * C:(h + 1) * C],
                                     wa_bcast[:])
                nc.scalar.activation(scr2[:], scr[:], AF.Copy,
                                     accum_out=s_cur[:])
                yp = ypsum.tile([P, 1], f32)
                nc.tensor.matmul(yp[:], lhsT=m1_sb[:], rhs=s_cur[:],
                                 start=True, stop=False)
                carry = m2h_sb if i == 0 else m2_sb
                nc.tensor.matmul(yp[:], lhsT=carry[:], rhs=s_prev[:],
                                 start=False, stop=True)
                ysb = spool.tile([P, 1], f32, tag="ycol")
                nc.vector.tensor_copy(ysb[:], yp[:])
                # out tile on DVE tensor_scalar (4x bf16 tier, ~0.7us) to keep
                # ACT free for the accum reductions
                nc.vector.tensor_scalar_mul(oc[:, h * C:(h + 1) * C],
                                            wp_bcast[:], ysb[:, 0:1])
                s_prev = s_cur
                if ch == NXC - 1:
                    # last chunk: drain per-tile so the final writes overlap
                    # the tail of the compute chain instead of one 2MB DMA
                    # after it
                    nc.gpsimd.dma_start(
                        out.ap()[i * P:(i + 1) * P, :],
                        oc[:, h * C:(h + 1) * C])
            if ch != NXC - 1:
                nc.gpsimd.dma_start(
                    out.ap()[ch * XCH * P:(ch + 1) * XCH * P, :]
                    .rearrange("(h p) c -> p h c", p=P),
                    oc[:].rearrange("p (h c) -> p h c", h=XCH))

    nc.compile()
    return nc


def _get_nc():
    if "nc" not in _BUILT:
        _BUILT["nc"] = _build_nc()
    return _BUILT["nc"]


def make_in_maps(x, W_attn, W_proj):
    import ml_dtypes
    bf = ml_dtypes.bfloat16
    x = np.asarray(x, dtype=np.float32)
    wa_bf = np.ascontiguousarray(np.asarray(W_attn, dtype=np.float32)).astype(bf)
    wpT_bf = np.ascontiguousarray(
        np.asarray(W_proj, dtype=np.float32).T).astype(bf)
    x_bf = x.astype(bf)
    m1_np, m2_np, m2h_np = _band_consts()
    consts = {
        "m1_band": m1_np, "m2_band": m2_np, "m2h_band": m2h_np,
        "ones_k": np.ones((P, 1), bf),
        "ones_m": np.ones((1, P), np.float32),
        "w_attn": wa_bf,
        "w_projT": wpT_bf,
    }
    in_maps = []
    for k in range(N_CORES):
        b, h = divmod(k, 2)
        t0 = h * T_LOC
        if h == 0:
            halo = np.zeros((2, C), bf)
        else:
            halo = np.ascontiguousarray(x_bf[b, t0 - 2:t0, :])
        in_maps.append({
            "x_shard": np.ascontiguousarray(x_bf[b, t0:t0 + T_LOC, :]),
            "x_halo": halo,
            **consts,
        })
    return in_maps


def assemble(results):
    out_full = np.empty((B, T, C), np.float32)
    for k in range(N_CORES):
        b, h = divmod(k, 2)
        t0 = h * T_LOC
        out_full[b, t0:t0 + T_LOC, :] = np.asarray(
            results[k]["out"], dtype=np.float32)
    return out_full


def kernel(x, W_attn, W_proj):
    from concourse.bass_utils import run_bass_kernel_spmd

    nc = _get_nc()
    in_maps = make_in_maps(x, W_attn, W_proj)
    res = run_bass_kernel_spmd(nc, in_maps, list(range(N_CORES)))
    return assemble(res.results)
